# revision 9
# baseline (speedup 1.0000x reference)
"""Multi-head attention (B=8, H=8, S=1024, d=128) on 8 TRN2 NeuronCores.

Strategy (v2)
-------------
- Tensor-parallel over heads: core c computes head c for ALL 8 batches.
  Each batch b is a "slot" whose key count is compacted + padded to its
  OWN 128-multiple (kt_b tiles), so total work is sum(kt_b) tiles
  instead of 8*max(kt_b) under batch-parallel sharding.
- Host-side prep (layout only): per (batch, head) compact keys/values
  to the seq_mask-selected rows, pre-transpose Q and K so the
  contraction dim lands on SBUF partitions, pre-tile V/ind so every
  DMA is row-contiguous, cast matmul operands to fp16. A tiny
  indicator matrix ind[k, 4] (1 for real keys) gives the softmax
  denominator via an M=4 matmul pair.
- Device: ONE flat software-pipelined stream over (slot, k-tile)
  iterations i, crossing slot boundaries (no per-head barrier):
    QK(i+1) emitted ahead ->  logitsT[k,q] = K^T.T @ Q^T   (PE)
    wt(i) = exp(logitsT * d^-0.5)                          (ACT, ->fp16)
    den(i), outT(i) accumulate via ind^T @ wt, V^T @ wt    (PE)
  All matmuls are column-tiled M=64 (den: M=4) diagonal pairs on
  disjoint PE column groups + disjoint PSUM banks so pairs co-execute.
  The learned scalar bias b cancels in softmax; -1e30 masking ==
  dropping masked keys, which the compaction does exactly.
- PSUM budget (8 banks): pl 2 bufs x 2 banks + po 2 + pd 2 = 8.
- Stores ride the gpsimd queue so they never block input loads on the
  sync queue. Numerator ships fp16 (halves output DMA); division and
  the fully-masked-batch fallback (uniform average) happen on host.
"""
from contextlib import ExitStack

import numpy as np

import concourse.bacc as bacc
import concourse.mybir as mybir
import concourse.tile as tile
from concourse.bass_utils import run_bass_kernel_spmd

F32 = mybir.dt.float32
F16 = mybir.dt.float16

B, S, D, H = 8, 1024, 1024, 8
DH = D // H              # 128, head dim = one partition tile
SCALE = float(DH) ** -0.5
NSLOT = 8                # slots per core = batches (head = core id)
IW = 4                   # indicator columns (den matmul M)

_NC_CACHE: dict[tuple, object] = {}

OPTS: dict = {}


def _build(kts: tuple[int, ...], opts: dict | None = None):
    """Build + compile the per-core kernel.

    kts[j] = number of 128-wide key tiles for slot j (descending)."""
    opts = opts or {}
    ktmax = max(kts)
    tot = sum(kts)
    offs = [sum(kts[:j]) for j in range(len(kts))]   # tile offset per slot
    nc = bacc.Bacc("TRN2", target_bir_lowering=False, debug=False)

    q_t = nc.dram_tensor("q_t", [NSLOT, DH, S], F16, kind="ExternalInput")
    k_t = nc.dram_tensor("k_t", [NSLOT, DH, ktmax * 128], F16,
                         kind="ExternalInput")
    v_c = nc.dram_tensor("v_c", [NSLOT, 128, ktmax * DH], F16,
                         kind="ExternalInput")
    ind = nc.dram_tensor("ind", [128, tot * IW], F16, kind="ExternalInput")
    out_t = nc.dram_tensor("out_t", [NSLOT, DH, S], F16, kind="ExternalOutput")
    den_t = nc.dram_tensor("den_t", [NSLOT, 2, S], F32, kind="ExternalOutput")

    with tile.TileContext(nc) as tc, ExitStack() as ctx:
        sb_q = ctx.enter_context(tc.tile_pool(name="sb_q", bufs=4))
        sb_k = ctx.enter_context(tc.tile_pool(name="sb_k", bufs=4))
        sb_v = ctx.enter_context(tc.tile_pool(name="sb_v", bufs=4))
        sb_ind = ctx.enter_context(tc.tile_pool(name="sb_ind", bufs=1))
        sb_w = ctx.enter_context(tc.tile_pool(name="sb_w", bufs=4))
        sb_out = ctx.enter_context(tc.tile_pool(name="sb_out", bufs=2))
        ps_l = ctx.enter_context(tc.tile_pool(name="ps_l", bufs=2, space="PSUM"))
        ps_o = ctx.enter_context(tc.tile_pool(name="ps_o", bufs=1, space="PSUM"))
        ps_d = ctx.enter_context(tc.tile_pool(name="ps_d", bufs=1, space="PSUM"))

        store = nc.gpsimd if opts.get("store_eng", "gpsimd") == "gpsimd" else nc.sync

        # ---- input DMA issue (sync queue), slot 0 split for fast start ----
        kth, qth, vh = {}, {}, {}

        def load_kq(j, split):
            kth[j] = sb_k.tile([128, kts[j] * 128], F16, tag="kth",
                               name=f"kth_{j}")
            qth[j] = sb_q.tile([128, S], F16, tag="qth", name=f"qth_{j}")
            if split:
                nc.sync.dma_start(kth[j][:, 0:128], k_t.ap()[j, :, 0:128])
                nc.sync.dma_start(qth[j][:, 0:512], q_t.ap()[j, :, 0:512])
                if kts[j] > 1:
                    nc.sync.dma_start(kth[j][:, 128:],
                                      k_t.ap()[j, :, 128:kts[j] * 128])
                nc.sync.dma_start(qth[j][:, 512:], q_t.ap()[j, :, 512:])
            else:
                nc.sync.dma_start(kth[j][:], k_t.ap()[j, :, 0:kts[j] * 128])
                nc.sync.dma_start(qth[j][:], q_t.ap()[j])

        def load_v(j):
            vh[j] = sb_v.tile([128, kts[j] * DH], F16, tag="vh",
                              name=f"vh_{j}")
            nc.sync.dma_start(vh[j][:], v_c.ap()[j, :, 0:kts[j] * DH])

        load_kq(0, split=True)
        ind_sb = sb_ind.tile([128, tot * IW], F16, name="ind_sb")
        nc.sync.dma_start(ind_sb[:], ind.ap())
        load_v(0)
        load_kq(1, split=False)

        # ---- flat (slot, tile) iteration stream ----
        iters = [(j, t) for j in range(NSLOT) for t in range(kts[j])]
        s0, s1 = slice(0, 512), slice(512, 1024)
        wts = {}
        po = {}
        pd = {}

        def emit_qk(i):
            j, t = iters[i]
            pl = ps_l.tile([128, S], F32, tag="pl", name=f"pl_{j}_{t}")
            ks = t * 128
            kA, kB = slice(ks, ks + 64), slice(ks + 64, ks + 128)
            nc.tensor.matmul(pl[0:64, s0], kth[j][:, kA], qth[j][:, s0])
            nc.tensor.matmul(pl[64:128, s1], kth[j][:, kB], qth[j][:, s1])
            nc.tensor.matmul(pl[64:128, s0], kth[j][:, kB], qth[j][:, s0])
            nc.tensor.matmul(pl[0:64, s1], kth[j][:, kA], qth[j][:, s1])
            wt = sb_w.tile([128, S], F16, tag="wt", name=f"wt_{j}_{t}")
            nc.scalar.activation(
                wt[:], pl[:], mybir.ActivationFunctionType.Exp, scale=SCALE
            )
            wts[i] = wt

        emit_qk(0)
        for i, (j, t) in enumerate(iters):
            # prefetch: next slot's K/Q two slots ahead, V one slot ahead
            if t == 0:
                if j + 2 < NSLOT:
                    load_kq(j + 2, split=False)
                if j + 1 < NSLOT:
                    load_v(j + 1)
            if i + 1 < len(iters):
                emit_qk(i + 1)
            wt = wts.pop(i)
            if t == 0:
                po[j] = ps_o.tile([128, S], F32, tag="po", name=f"po_{j}")
                pd[j] = ps_d.tile([64, S], F32, tag="pd", name=f"pd_{j}")
            ks = t * 128
            dA, dB = slice(ks, ks + 64), slice(ks + 64, ks + 128)
            first, last = t == 0, t == kts[j] - 1
            ic = slice((offs[j] + t) * IW, (offs[j] + t) * IW + IW)
            mms = [
                (pd[j][0:IW, s0], ind_sb[:, ic], wt[:, s0]),
                (pd[j][32:32 + IW, s1], ind_sb[:, ic], wt[:, s1]),
                (po[j][0:64, s0], vh[j][:, dA], wt[:, s0]),
                (po[j][64:128, s1], vh[j][:, dB], wt[:, s1]),
                (po[j][64:128, s0], vh[j][:, dB], wt[:, s0]),
                (po[j][0:64, s1], vh[j][:, dA], wt[:, s1]),
            ]
            for out_ap, w_ap, r_ap in mms:
                nc.tensor.matmul(out_ap, w_ap, r_ap, start=first, stop=last)

            if last:
                # denominator rows 0 (q-chunk 0) and 32 (q-chunk 1)
                dsb = sb_out.tile([33, S], F32, tag="dsb", name=f"dsb_{j}")
                nc.vector.tensor_copy(dsb[0:1, :], pd[j][0:1, :])
                nc.vector.tensor_copy(dsb[32:33, :], pd[j][32:33, :])
                store.dma_start(den_t.ap()[j, 0:1, :], dsb[0:1, :])
                store.dma_start(den_t.ap()[j, 1:2, :], dsb[32:33, :])
                osb = sb_out.tile([128, S], F16, tag="osb", name=f"osb_{j}")
                nc.vector.tensor_copy(osb[:], po[j][:])
                store.dma_start(out_t.ap()[j, :, :], osb[:])

    nc.compile()
    return nc


def kernel(memory, query, seq_mask, b):
    memory = np.ascontiguousarray(memory, dtype=np.float32)
    query = np.ascontiguousarray(query, dtype=np.float32)
    seq_mask = np.asarray(seq_mask)
    assert memory.shape == (B, S, 2 * D) and query.shape == (B, S, D)

    counts = [int(np.count_nonzero(seq_mask[i])) for i in range(B)]
    tiles = [max(1, -(-c // 128)) for c in counts]
    perm = sorted(range(B), key=lambda i: -tiles[i])   # slot j <- batch perm[j]
    kts = tuple(tiles[p] for p in perm)
    ktmax, tot = max(kts), sum(kts)
    offs = [sum(kts[:j]) for j in range(B)]

    key = (kts, tuple(sorted(OPTS.items())))
    if key not in _NC_CACHE:
        _NC_CACHE[key] = _build(kts, OPTS)
    nc = _NC_CACHE[key]

    # indicator: shared across cores. ind[p, (off_j+t)*IW + i] = 1 iff
    # key t*128+p of slot j is real.
    ind = np.zeros((128, tot * IW), dtype=np.float16)
    # per-(slot, head) operand tensors, one in_map per core (head = core)
    in_maps = [
        {"q_t": np.zeros((NSLOT, DH, S), np.float16),
         "k_t": np.zeros((NSLOT, DH, ktmax * 128), np.float16),
         "v_c": np.zeros((NSLOT, 128, ktmax * DH), np.float16),
         "ind": ind}
        for _ in range(H)
    ]
    for j, bidx in enumerate(perm):
        idx = np.flatnonzero(seq_mask[bidx])
        nb = len(idx)
        kp = kts[j] * 128
        if nb:
            kc = memory[bidx, idx, :D].astype(np.float16)        # [nb, D]
            vc = memory[bidx, idx, D:].astype(np.float16)        # [nb, D]
            qc = query[bidx].astype(np.float16)                  # [S, D]
            vtile = np.zeros((kp, D), np.float16)
            vtile[:nb] = vc
            # [kp, D] -> [kt, 128, H, DH] -> per head [128, kt*DH]
            vtile = vtile.reshape(kts[j], 128, H, DH)
            m = np.zeros((128, kts[j] * IW), np.float16)
            for t in range(kts[j]):
                lo, hi = t * 128, min(nb, (t + 1) * 128)
                if hi > lo:
                    m[0:hi - lo, t * IW:(t + 1) * IW] = 1.0
            ind[:, offs[j] * IW:(offs[j] + kts[j]) * IW] = m
            for c in range(H):
                hs = c * DH
                im = in_maps[c]
                im["q_t"][j] = qc[:, hs:hs + DH].T
                im["k_t"][j, :, :nb] = kc[:, hs:hs + DH].T
                im["v_c"][j, :, :kts[j] * DH] = (
                    vtile[:, :, c, :].transpose(1, 0, 2).reshape(128, -1))

    res = run_bass_kernel_spmd(nc, in_maps, list(range(H)))

    out = np.empty((B, S, D), dtype=np.float32)
    for c in range(H):
        num = res.results[c]["out_t"].astype(np.float32)   # [NSLOT, DH, S]
        dd = res.results[c]["den_t"]                       # [NSLOT, 2, S]
        for j, bidx in enumerate(perm):
            den = np.concatenate([dd[j, 0, 0:512], dd[j, 1, 512:1024]])
            with np.errstate(divide="ignore", invalid="ignore"):
                out[bidx, :, c * DH:(c + 1) * DH] = (num[j] / den[None, :]).T
    for bidx in range(B):
        if counts[bidx] == 0:
            # all keys masked: reference softmax degenerates to uniform
            out[bidx] = memory[bidx, :, D:].mean(axis=0)[None, :]
    return out


# revision 12
# speedup vs baseline: 1.2638x; 1.2638x over previous
"""Multi-head attention (B=8, H=8, S=1024, d=128) on 8 TRN2 NeuronCores.

Strategy (v2)
-------------
- Tensor-parallel over heads: core c computes head c for ALL 8 batches.
  Each batch b is a "slot" whose key count is compacted + padded to its
  OWN 128-multiple (kt_b tiles), so total work is sum(kt_b) tiles
  instead of 8*max(kt_b) under batch-parallel sharding.
- Host-side prep (layout only): per (batch, head) compact keys/values
  to the seq_mask-selected rows, pre-transpose Q and K so the
  contraction dim lands on SBUF partitions, pre-tile V/ind so every
  DMA is row-contiguous, cast matmul operands to fp16. A tiny
  indicator matrix ind[k, 4] (1 for real keys) gives the softmax
  denominator via an M=4 matmul pair.
- Device: ONE flat software-pipelined stream over (slot, k-tile)
  iterations i, crossing slot boundaries (no per-head barrier):
    QK(i+1) emitted ahead ->  logitsT[k,q] = K^T.T @ Q^T   (PE)
    wt(i) = exp(logitsT * d^-0.5)                          (ACT, ->fp16)
    den(i), outT(i) accumulate via ind^T @ wt, V^T @ wt    (PE)
  All matmuls are column-tiled M=64 (den: M=4) diagonal pairs on
  disjoint PE column groups + disjoint PSUM banks so pairs co-execute.
  The learned scalar bias b cancels in softmax; -1e30 masking ==
  dropping masked keys, which the compaction does exactly.
- PSUM budget (8 banks): pl 2 bufs x 2 banks + po 2 + pd 2 = 8.
- Stores ride the gpsimd queue so they never block input loads on the
  sync queue. Numerator ships fp16 (halves output DMA); division and
  the fully-masked-batch fallback (uniform average) happen on host.
"""
from contextlib import ExitStack

import numpy as np

import concourse.bacc as bacc
import concourse.mybir as mybir
import concourse.tile as tile
from concourse.bass_utils import run_bass_kernel_spmd

F32 = mybir.dt.float32
F16 = mybir.dt.float16

B, S, D, H = 8, 1024, 1024, 8
DH = D // H              # 128, head dim = one partition tile
SCALE = float(DH) ** -0.5
NSLOT = 8                # slots per core = batches (head = core id)
IW = 4                   # indicator columns (den matmul M)

_NC_CACHE: dict[tuple, object] = {}

OPTS: dict = {}


def _build(kts: tuple[int, ...], opts: dict | None = None):
    """Build + compile the per-core kernel.

    kts[j] = number of 128-wide key tiles for slot j (descending)."""
    opts = opts or {}
    ktmax = max(kts)
    tot = sum(kts)
    offs = [sum(kts[:j]) for j in range(len(kts))]   # tile offset per slot
    nc = bacc.Bacc("TRN2", target_bir_lowering=False, debug=False)

    q_t = nc.dram_tensor("q_t", [NSLOT, DH, S], F16, kind="ExternalInput")
    k_t = nc.dram_tensor("k_t", [NSLOT, DH, ktmax * 128], F16,
                         kind="ExternalInput")
    v_c = nc.dram_tensor("v_c", [NSLOT, 128, ktmax * DH], F16,
                         kind="ExternalInput")
    ind = nc.dram_tensor("ind", [128, tot * IW], F16, kind="ExternalInput")
    out_t = nc.dram_tensor("out_t", [NSLOT, DH, S], F16, kind="ExternalOutput")
    den_t = nc.dram_tensor("den_t", [NSLOT, 2, S], F32, kind="ExternalOutput")

    with tile.TileContext(nc) as tc, ExitStack() as ctx:
        sb_q = ctx.enter_context(tc.tile_pool(name="sb_q", bufs=4))
        sb_k = ctx.enter_context(tc.tile_pool(name="sb_k", bufs=4))
        sb_v = ctx.enter_context(tc.tile_pool(name="sb_v", bufs=4))
        sb_ind = ctx.enter_context(tc.tile_pool(name="sb_ind", bufs=1))
        sb_w = ctx.enter_context(tc.tile_pool(name="sb_w", bufs=4))
        sb_out = ctx.enter_context(tc.tile_pool(name="sb_out", bufs=2))
        ps_l = ctx.enter_context(tc.tile_pool(name="ps_l", bufs=2, space="PSUM"))
        ps_o = ctx.enter_context(tc.tile_pool(name="ps_o", bufs=1, space="PSUM"))
        ps_d = ctx.enter_context(tc.tile_pool(name="ps_d", bufs=1, space="PSUM"))

        store = nc.gpsimd if opts.get("store_eng", "sync") == "gpsimd" else nc.sync

        # ---- input DMA issue (sync queue), slot 0 split for fast start ----
        kth, qth, vh = {}, {}, {}

        def load_kq(j, split):
            kth[j] = sb_k.tile([128, kts[j] * 128], F16, tag="kth",
                               name=f"kth_{j}")
            qth[j] = sb_q.tile([128, S], F16, tag="qth", name=f"qth_{j}")
            if split:
                nc.sync.dma_start(kth[j][:, 0:128], k_t.ap()[j, :, 0:128])
                nc.sync.dma_start(qth[j][:, 0:512], q_t.ap()[j, :, 0:512])
                if kts[j] > 1:
                    nc.sync.dma_start(kth[j][:, 128:],
                                      k_t.ap()[j, :, 128:kts[j] * 128])
                nc.sync.dma_start(qth[j][:, 512:], q_t.ap()[j, :, 512:])
            else:
                nc.sync.dma_start(kth[j][:], k_t.ap()[j, :, 0:kts[j] * 128])
                nc.sync.dma_start(qth[j][:], q_t.ap()[j])

        def load_v(j):
            vh[j] = sb_v.tile([128, kts[j] * DH], F16, tag="vh",
                              name=f"vh_{j}")
            nc.sync.dma_start(vh[j][:], v_c.ap()[j, :, 0:kts[j] * DH])

        load_kq(0, split=True)
        ind_sb = sb_ind.tile([128, tot * IW], F16, name="ind_sb")
        nc.sync.dma_start(ind_sb[:], ind.ap())
        load_v(0)
        load_kq(1, split=False)

        # ---- flat (slot, tile) iteration stream ----
        iters = [(j, t) for j in range(NSLOT) for t in range(kts[j])]
        s0, s1 = slice(0, 512), slice(512, 1024)
        wts = {}
        po = {}
        pd = {}

        def emit_qk(i):
            j, t = iters[i]
            pl = ps_l.tile([128, S], F32, tag="pl", name=f"pl_{j}_{t}")
            ks = t * 128
            kA, kB = slice(ks, ks + 64), slice(ks + 64, ks + 128)
            nc.tensor.matmul(pl[0:64, s0], kth[j][:, kA], qth[j][:, s0])
            nc.tensor.matmul(pl[64:128, s1], kth[j][:, kB], qth[j][:, s1])
            nc.tensor.matmul(pl[64:128, s0], kth[j][:, kB], qth[j][:, s0])
            nc.tensor.matmul(pl[0:64, s1], kth[j][:, kA], qth[j][:, s1])
            wt = sb_w.tile([128, S], F16, tag="wt", name=f"wt_{j}_{t}")
            nc.scalar.activation(
                wt[:], pl[:], mybir.ActivationFunctionType.Exp, scale=SCALE
            )
            wts[i] = wt

        emit_qk(0)
        if len(iters) > 1:
            emit_qk(1)
        for i, (j, t) in enumerate(iters):
            # prefetch: next slot's K/Q two slots ahead, V one slot ahead
            if t == 0:
                if j + 2 < NSLOT:
                    load_kq(j + 2, split=False)
                if j + 1 < NSLOT:
                    load_v(j + 1)
            if i + 2 < len(iters):
                emit_qk(i + 2)
            wt = wts.pop(i)
            if t == 0:
                po[j] = ps_o.tile([128, S], F32, tag="po", name=f"po_{j}")
                pd[j] = ps_d.tile([64, S], F32, tag="pd", name=f"pd_{j}")
            ks = t * 128
            dA, dB = slice(ks, ks + 64), slice(ks + 64, ks + 128)
            first, last = t == 0, t == kts[j] - 1
            ic = slice((offs[j] + t) * IW, (offs[j] + t) * IW + IW)
            mms = [
                (pd[j][0:IW, s0], ind_sb[:, ic], wt[:, s0]),
                (pd[j][32:32 + IW, s1], ind_sb[:, ic], wt[:, s1]),
                (po[j][0:64, s0], vh[j][:, dA], wt[:, s0]),
                (po[j][64:128, s1], vh[j][:, dB], wt[:, s1]),
                (po[j][64:128, s0], vh[j][:, dB], wt[:, s0]),
                (po[j][0:64, s1], vh[j][:, dA], wt[:, s1]),
            ]
            for out_ap, w_ap, r_ap in mms:
                nc.tensor.matmul(out_ap, w_ap, r_ap, start=first, stop=last)

            if last:
                # denominator rows 0 (q-chunk 0) and 32 (q-chunk 1)
                dsb = sb_out.tile([33, S], F32, tag="dsb", name=f"dsb_{j}")
                nc.vector.tensor_copy(dsb[:], pd[j][0:33, :])
                store.dma_start(den_t.ap()[j, 0:1, :], dsb[0:1, :])
                store.dma_start(den_t.ap()[j, 1:2, :], dsb[32:33, :])
                osb = sb_out.tile([128, S], F16, tag="osb", name=f"osb_{j}")
                nc.vector.tensor_copy(osb[:], po[j][:])
                store.dma_start(out_t.ap()[j, :, :], osb[:])

    nc.compile()
    return nc


def kernel(memory, query, seq_mask, b):
    memory = np.ascontiguousarray(memory, dtype=np.float32)
    query = np.ascontiguousarray(query, dtype=np.float32)
    seq_mask = np.asarray(seq_mask)
    assert memory.shape == (B, S, 2 * D) and query.shape == (B, S, D)

    counts = [int(np.count_nonzero(seq_mask[i])) for i in range(B)]
    tiles = [max(1, -(-c // 128)) for c in counts]
    perm = sorted(range(B), key=lambda i: -tiles[i])   # slot j <- batch perm[j]
    kts = tuple(tiles[p] for p in perm)
    ktmax, tot = max(kts), sum(kts)
    offs = [sum(kts[:j]) for j in range(B)]

    key = (kts, tuple(sorted(OPTS.items())))
    if key not in _NC_CACHE:
        _NC_CACHE[key] = _build(kts, OPTS)
    nc = _NC_CACHE[key]

    # indicator: shared across cores. ind[p, (off_j+t)*IW + i] = 1 iff
    # key t*128+p of slot j is real.
    ind = np.zeros((128, tot * IW), dtype=np.float16)
    # per-(slot, head) operand tensors, one in_map per core (head = core)
    in_maps = [
        {"q_t": np.zeros((NSLOT, DH, S), np.float16),
         "k_t": np.zeros((NSLOT, DH, ktmax * 128), np.float16),
         "v_c": np.zeros((NSLOT, 128, ktmax * DH), np.float16),
         "ind": ind}
        for _ in range(H)
    ]
    for j, bidx in enumerate(perm):
        idx = np.flatnonzero(seq_mask[bidx])
        nb = len(idx)
        kp = kts[j] * 128
        if nb:
            kc = memory[bidx, idx, :D].astype(np.float16)        # [nb, D]
            vc = memory[bidx, idx, D:].astype(np.float16)        # [nb, D]
            qc = query[bidx].astype(np.float16)                  # [S, D]
            vtile = np.zeros((kp, D), np.float16)
            vtile[:nb] = vc
            # [kp, D] -> [kt, 128, H, DH] -> per head [128, kt*DH]
            vtile = vtile.reshape(kts[j], 128, H, DH)
            m = np.zeros((128, kts[j] * IW), np.float16)
            for t in range(kts[j]):
                lo, hi = t * 128, min(nb, (t + 1) * 128)
                if hi > lo:
                    m[0:hi - lo, t * IW:(t + 1) * IW] = 1.0
            ind[:, offs[j] * IW:(offs[j] + kts[j]) * IW] = m
            for c in range(H):
                hs = c * DH
                im = in_maps[c]
                im["q_t"][j] = qc[:, hs:hs + DH].T
                im["k_t"][j, :, :nb] = kc[:, hs:hs + DH].T
                im["v_c"][j, :, :kts[j] * DH] = (
                    vtile[:, :, c, :].transpose(1, 0, 2).reshape(128, -1))

    res = run_bass_kernel_spmd(nc, in_maps, list(range(H)))

    out = np.empty((B, S, D), dtype=np.float32)
    for c in range(H):
        num = res.results[c]["out_t"].astype(np.float32)   # [NSLOT, DH, S]
        dd = res.results[c]["den_t"]                       # [NSLOT, 2, S]
        for j, bidx in enumerate(perm):
            den = np.concatenate([dd[j, 0, 0:512], dd[j, 1, 512:1024]])
            with np.errstate(divide="ignore", invalid="ignore"):
                out[bidx, :, c * DH:(c + 1) * DH] = (num[j] / den[None, :]).T
    for bidx in range(B):
        if counts[bidx] == 0:
            # all keys masked: reference softmax degenerates to uniform
            out[bidx] = memory[bidx, :, D:].mean(axis=0)[None, :]
    return out


# revision 17
# speedup vs baseline: 1.3176x; 1.0426x over previous
"""Multi-head attention (B=8, H=8, S=1024, d=128) on 8 TRN2 NeuronCores.

Strategy (v2)
-------------
- Tensor-parallel over heads: core c computes head c for ALL 8 batches.
  Each batch b is a "slot" whose key count is compacted + padded to its
  OWN 128-multiple (kt_b tiles), so total work is sum(kt_b) tiles
  instead of 8*max(kt_b) under batch-parallel sharding.
- Host-side prep (layout only): per (batch, head) compact keys/values
  to the seq_mask-selected rows, pre-transpose Q and K so the
  contraction dim lands on SBUF partitions, pre-tile V/ind so every
  DMA is row-contiguous, cast matmul operands to fp16. A tiny
  indicator matrix ind[k, 4] (1 for real keys) gives the softmax
  denominator via an M=4 matmul pair.
- Device: ONE flat software-pipelined stream over (slot, k-tile)
  iterations i, crossing slot boundaries (no per-head barrier):
    QK(i+1) emitted ahead ->  logitsT[k,q] = K^T.T @ Q^T   (PE)
    wt(i) = exp(logitsT * d^-0.5)                          (ACT, ->fp16)
    den(i), outT(i) accumulate via ind^T @ wt, V^T @ wt    (PE)
  All matmuls are column-tiled M=64 (den: M=4) diagonal pairs on
  disjoint PE column groups + disjoint PSUM banks so pairs co-execute.
  The learned scalar bias b cancels in softmax; -1e30 masking ==
  dropping masked keys, which the compaction does exactly.
- PSUM budget (8 banks): pl 2 bufs x 2 banks + po 2 + pd 2 = 8.
- Stores ride the gpsimd queue so they never block input loads on the
  sync queue. Numerator ships fp16 (halves output DMA); division and
  the fully-masked-batch fallback (uniform average) happen on host.
"""
from contextlib import ExitStack

import numpy as np

import concourse.bacc as bacc
import concourse.mybir as mybir
import concourse.tile as tile
from concourse.bass_utils import run_bass_kernel_spmd

F32 = mybir.dt.float32
F16 = mybir.dt.float16

B, S, D, H = 8, 1024, 1024, 8
DH = D // H              # 128, head dim = one partition tile
SCALE = float(DH) ** -0.5
NSLOT = 8                # slots per core = batches (head = core id)
IW = 4                   # indicator columns (den matmul M)

_NC_CACHE: dict[tuple, object] = {}

OPTS: dict = {}


def _build(kts: tuple[int, ...], opts: dict | None = None):
    """Build + compile the per-core kernel.

    kts[j] = number of 128-wide key tiles for slot j (descending)."""
    opts = opts or {}
    ktmax = max(kts)
    tot = sum(kts)
    offs = [sum(kts[:j]) for j in range(len(kts))]   # tile offset per slot
    nc = bacc.Bacc("TRN2", target_bir_lowering=False, debug=False)

    q_t = nc.dram_tensor("q_t", [NSLOT, DH, S], F16, kind="ExternalInput")
    k_t = nc.dram_tensor("k_t", [NSLOT, DH, ktmax * 128], F16,
                         kind="ExternalInput")
    v_c = nc.dram_tensor("v_c", [NSLOT, 128, ktmax * DH], F16,
                         kind="ExternalInput")
    ind = nc.dram_tensor("ind", [128, tot * IW], F16, kind="ExternalInput")
    out_t = nc.dram_tensor("out_t", [NSLOT, DH, S], F16, kind="ExternalOutput")
    den_t = nc.dram_tensor("den_t", [NSLOT, 2, 512], F32, kind="ExternalOutput")

    with tile.TileContext(nc) as tc, ExitStack() as ctx:
        sb_q = ctx.enter_context(tc.tile_pool(name="sb_q", bufs=4))
        sb_k = ctx.enter_context(tc.tile_pool(name="sb_k", bufs=4))
        sb_v = ctx.enter_context(tc.tile_pool(name="sb_v", bufs=4))
        sb_ind = ctx.enter_context(tc.tile_pool(name="sb_ind", bufs=1))
        sb_w = ctx.enter_context(tc.tile_pool(name="sb_w", bufs=4))
        sb_out = ctx.enter_context(tc.tile_pool(name="sb_out", bufs=2))
        ps_l = ctx.enter_context(tc.tile_pool(name="ps_l", bufs=2, space="PSUM"))
        ps_o0 = ctx.enter_context(tc.tile_pool(name="ps_o0", bufs=2, space="PSUM"))
        ps_o1 = ctx.enter_context(tc.tile_pool(name="ps_o1", bufs=1, space="PSUM"))
        ps_d = ctx.enter_context(tc.tile_pool(name="ps_d", bufs=1, space="PSUM"))

        store = nc.gpsimd if opts.get("store_eng", "sync") == "gpsimd" else nc.sync

        # PE warmup: dense dummy matmuls during the initial DMA wait so the
        # HAM clock gate is at 8/8 (2.4 GHz) when the first real QK lands.
        wm = sb_ind.tile([128, 576], F16, name="wm")
        nc.gpsimd.memset(wm[:], 0.0)
        warm_ps = ps_l.tile([128, 512], F32, tag="pl", name="warm_ps")
        for _ in range(opts.get("warm_mms", 6)):
            nc.tensor.matmul(warm_ps[0:64, :], wm[:, 0:64], wm[:, 64:])

        # ---- input DMA issue (sync queue), slot 0 split for fast start ----
        kth, qth, vh = {}, {}, {}

        def load_kq(j, split):
            kth[j] = sb_k.tile([128, kts[j] * 128], F16, tag="kth",
                               name=f"kth_{j}")
            qth[j] = sb_q.tile([128, S], F16, tag="qth", name=f"qth_{j}")
            if split:
                nc.sync.dma_start(kth[j][:, 0:128], k_t.ap()[j, :, 0:128])
                nc.sync.dma_start(qth[j][:, 0:512], q_t.ap()[j, :, 0:512])
                if kts[j] > 1:
                    nc.sync.dma_start(kth[j][:, 128:],
                                      k_t.ap()[j, :, 128:kts[j] * 128])
                nc.sync.dma_start(qth[j][:, 512:], q_t.ap()[j, :, 512:])
            else:
                nc.sync.dma_start(kth[j][:], k_t.ap()[j, :, 0:kts[j] * 128])
                nc.sync.dma_start(qth[j][:], q_t.ap()[j])

        def load_v(j):
            vh[j] = sb_v.tile([128, kts[j] * DH], F16, tag="vh",
                              name=f"vh_{j}")
            nc.sync.dma_start(vh[j][:], v_c.ap()[j, :, 0:kts[j] * DH])

        load_kq(0, split=True)
        ind_sb = sb_ind.tile([128, tot * IW], F16, name="ind_sb")
        nc.sync.dma_start(ind_sb[:], ind.ap())
        load_v(0)
        load_kq(1, split=False)

        # ---- flat (slot, tile) iteration stream ----
        iters = [(j, t) for j in range(NSLOT) for t in range(kts[j])]
        s0, s1 = slice(0, 512), slice(512, 1024)
        wts = {}
        po0 = {}
        po1 = {}
        pd = {}

        def emit_qk(i):
            j, t = iters[i]
            pl = ps_l.tile([128, S], F32, tag="pl", name=f"pl_{j}_{t}")
            ks = t * 128
            kA, kB = slice(ks, ks + 64), slice(ks + 64, ks + 128)
            nc.tensor.matmul(pl[0:64, s0], kth[j][:, kA], qth[j][:, s0])
            nc.tensor.matmul(pl[64:128, s1], kth[j][:, kB], qth[j][:, s1])
            nc.tensor.matmul(pl[64:128, s0], kth[j][:, kB], qth[j][:, s0])
            nc.tensor.matmul(pl[0:64, s1], kth[j][:, kA], qth[j][:, s1])
            wt = sb_w.tile([128, S], F16, tag="wt", name=f"wt_{j}_{t}")
            nc.scalar.activation(
                wt[:], pl[:], mybir.ActivationFunctionType.Exp, scale=SCALE
            )
            wts[i] = wt

        emit_qk(0)
        if len(iters) > 1:
            emit_qk(1)
        for i, (j, t) in enumerate(iters):
            # prefetch: next slot's K/Q two slots ahead, V one slot ahead
            if t == 0:
                if j + 2 < NSLOT:
                    load_kq(j + 2, split=False)
                if j + 1 < NSLOT:
                    load_v(j + 1)
            if i + 2 < len(iters):
                emit_qk(i + 2)
            wt = wts.pop(i)
            if t == 0:
                po0[j] = ps_o0.tile([128, 512], F32, tag="po0", name=f"po0_{j}")
                po1[j] = ps_o1.tile([128, 512], F32, tag="po1", name=f"po1_{j}")
                # den: q-chunk 0 at partitions 0:IW, q-chunk 1 at 32:32+IW,
                # both in the same single PSUM bank
                pd[j] = ps_d.tile([64, 512], F32, tag="pd", name=f"pd_{j}")
            ks = t * 128
            dA, dB = slice(ks, ks + 64), slice(ks + 64, ks + 128)
            first, last = t == 0, t == kts[j] - 1
            ic = slice((offs[j] + t) * IW, (offs[j] + t) * IW + IW)
            mms = [
                (pd[j][0:IW, :], ind_sb[:, ic], wt[:, s0]),
                (pd[j][32:32 + IW, :], ind_sb[:, ic], wt[:, s1]),
                (po0[j][0:64, :], vh[j][:, dA], wt[:, s0]),
                (po1[j][64:128, :], vh[j][:, dB], wt[:, s1]),
                (po0[j][64:128, :], vh[j][:, dB], wt[:, s0]),
                (po1[j][0:64, :], vh[j][:, dA], wt[:, s1]),
            ]
            for out_ap, w_ap, r_ap in mms:
                nc.tensor.matmul(out_ap, w_ap, r_ap, start=first, stop=last)

            if last:
                # denominator rows 0 (q-chunk 0) and 32 (q-chunk 1)
                dsb = sb_out.tile([33, 512], F32, tag="dsb", name=f"dsb_{j}")
                nc.vector.tensor_copy(dsb[:], pd[j][0:33, :])
                store.dma_start(den_t.ap()[j, 0:1, :], dsb[0:1, :])
                store.dma_start(den_t.ap()[j, 1:2, :], dsb[32:33, :])
                osb = sb_out.tile([128, S], F16, tag="osb", name=f"osb_{j}")
                nc.vector.tensor_copy(osb[:, s1], po1[j][:])
                nc.vector.tensor_copy(osb[:, s0], po0[j][:])
                store.dma_start(out_t.ap()[j, :, :], osb[:])

    nc.compile()
    return nc


def kernel(memory, query, seq_mask, b):
    memory = np.ascontiguousarray(memory, dtype=np.float32)
    query = np.ascontiguousarray(query, dtype=np.float32)
    seq_mask = np.asarray(seq_mask)
    assert memory.shape == (B, S, 2 * D) and query.shape == (B, S, D)

    counts = [int(np.count_nonzero(seq_mask[i])) for i in range(B)]
    tiles = [max(1, -(-c // 128)) for c in counts]
    perm = sorted(range(B), key=lambda i: -tiles[i])   # slot j <- batch perm[j]
    kts = tuple(tiles[p] for p in perm)
    ktmax, tot = max(kts), sum(kts)
    offs = [sum(kts[:j]) for j in range(B)]

    key = (kts, tuple(sorted(OPTS.items())))
    if key not in _NC_CACHE:
        _NC_CACHE[key] = _build(kts, OPTS)
    nc = _NC_CACHE[key]

    # indicator: shared across cores. ind[p, (off_j+t)*IW + i] = 1 iff
    # key t*128+p of slot j is real.
    ind = np.zeros((128, tot * IW), dtype=np.float16)
    # per-(slot, head) operand tensors, one in_map per core (head = core)
    in_maps = [
        {"q_t": np.zeros((NSLOT, DH, S), np.float16),
         "k_t": np.zeros((NSLOT, DH, ktmax * 128), np.float16),
         "v_c": np.zeros((NSLOT, 128, ktmax * DH), np.float16),
         "ind": ind}
        for _ in range(H)
    ]
    for j, bidx in enumerate(perm):
        idx = np.flatnonzero(seq_mask[bidx])
        nb = len(idx)
        kp = kts[j] * 128
        if nb:
            kc = memory[bidx, idx, :D].astype(np.float16)        # [nb, D]
            vc = memory[bidx, idx, D:].astype(np.float16)        # [nb, D]
            qc = query[bidx].astype(np.float16)                  # [S, D]
            vtile = np.zeros((kp, D), np.float16)
            vtile[:nb] = vc
            # [kp, D] -> [kt, 128, H, DH] -> per head [128, kt*DH]
            vtile = vtile.reshape(kts[j], 128, H, DH)
            m = np.zeros((128, kts[j] * IW), np.float16)
            for t in range(kts[j]):
                lo, hi = t * 128, min(nb, (t + 1) * 128)
                if hi > lo:
                    m[0:hi - lo, t * IW:(t + 1) * IW] = 1.0
            ind[:, offs[j] * IW:(offs[j] + kts[j]) * IW] = m
            for c in range(H):
                hs = c * DH
                im = in_maps[c]
                im["q_t"][j] = qc[:, hs:hs + DH].T
                im["k_t"][j, :, :nb] = kc[:, hs:hs + DH].T
                im["v_c"][j, :, :kts[j] * DH] = (
                    vtile[:, :, c, :].transpose(1, 0, 2).reshape(128, -1))

    res = run_bass_kernel_spmd(nc, in_maps, list(range(H)))

    out = np.empty((B, S, D), dtype=np.float32)
    for c in range(H):
        num = res.results[c]["out_t"].astype(np.float32)   # [NSLOT, DH, S]
        dd = res.results[c]["den_t"]                       # [NSLOT, 2, 512]
        for j, bidx in enumerate(perm):
            den = np.concatenate([dd[j, 0], dd[j, 1]])
            with np.errstate(divide="ignore", invalid="ignore"):
                out[bidx, :, c * DH:(c + 1) * DH] = (num[j] / den[None, :]).T
    for bidx in range(B):
        if counts[bidx] == 0:
            # all keys masked: reference softmax degenerates to uniform
            out[bidx] = memory[bidx, :, D:].mean(axis=0)[None, :]
    return out
